# revision 2
# baseline (speedup 1.0000x reference)
"""GCEncoder (RGCN basis-decomposition conv + mean aggregation + Dense/BN/ReLU)
as a Bass/Tile kernel on 8 Trainium2 NeuronCores.

Math (reference):
  W[r]  = sum_b comp[r,b] * basis[b]                    [R, N, H0]
  h[r]  = x @ W[r]                                      [R, N, H0]
  agg[d] = sum_r (1/cnt[d,r]) * sum_{e: dst=d, type=r} h[r, src_e]
  feats = agg + x @ root + bias
  z     = feats @ fc_w.T ; per-row batchnorm over H1 + gamma/beta + relu
  out   = (z[:U], z[U:]) stacked -> [2, U, H1]

Device strategy (per core c of 8, 512 node-rows each):
  Phase A: h rows for this core's 512 src rows: h_c = x[rows] @ Wall where
           Wall = [W[0] | ... | W[4] | root]  (4096 x 3000), fp32r matmuls.
           The root block result stays local (rows == this core's dst rows);
           the W blocks go to an AllGather -> full h (4096 x 2500) everywhere.
  Phase B: agg rows = AT_c^T-contraction: matmul over the 20480 (r,src) axis
           with a host-built dense normalized-adjacency slice
           AT[(r,src), dst_local] (fp32r), PSUM-accumulated over 160 k-tiles.
  Phase C: feats = agg + root_part + bias; PE-transpose; z = feats @ fc_w.T;
           per-row BN (bn_stats/bn_aggr) + gamma/beta + ReLU.

All fp32r operands are pre-rounded host-side to E8M11 (RNE at mantissa bit 12),
matching HW exactly (verified bit-exact).
"""
import numpy as np

import concourse.bacc as bacc
import concourse.mybir as mybir
import concourse.tile as tile
from concourse.bass_utils import run_bass_kernel_spmd
from concourse.masks import make_identity

P = 128
NCORES = 8
N = 4096          # nodes
U = 2048          # users
R = 5             # relations
B = 30            # bases
H0 = 500
H1 = 75
E = 262144
EPS = 1e-5

NL = N // NCORES              # 512 node rows per core
KB_A = N // P                 # 32 contraction tiles, phase A
WCOL = R * H0 + H0            # 3000 Wall columns
KB_B = R * KB_A               # 160 contraction tiles, phase B
NBLK = WCOL // H0             # 6 column blocks of 500
MB = NL // P                  # 4 M-tiles per core
QB = 4                        # H0 chunks of 125 for transpose/fc
QS = H0 // QB                 # 125

F32 = mybir.dt.float32
F32R = mybir.dt.float32r

# test hooks: set TRACE=True before calling kernel() to capture an NTFF
# profile; results land in LAST_RESULTS.
TRACE = False
LAST_RESULTS = None

_NC_CACHE = None


def round_fp32r(a: np.ndarray) -> np.ndarray:
    """Round fp32 to fp32r (E8M11): RNE at mantissa bit 12, low 12 bits zero."""
    b = np.ascontiguousarray(a, dtype=np.float32).view(np.uint32).astype(np.uint64)
    b = b + 0x7FF + ((b >> 12) & 1)
    return (b & 0xFFFFF000).astype(np.uint32).view(np.float32)


def _build():
    nc = bacc.Bacc("TRN2", target_bir_lowering=False, debug=False,
                   num_devices=NCORES)

    xt_d = nc.dram_tensor("xt", [N, NL], F32R, kind="ExternalInput")
    wall_d = nc.dram_tensor("wall", [N, WCOL], F32R, kind="ExternalInput")
    at_d = nc.dram_tensor("at", [R * N, NL], F32R, kind="ExternalInput")
    fcwt_d = nc.dram_tensor("fcwt", [H0, H1], F32, kind="ExternalInput")
    biasb_d = nc.dram_tensor("biasb", [P, H0], F32, kind="ExternalInput")
    gamma_d = nc.dram_tensor("gamma", [P, MB], F32, kind="ExternalInput")
    beta_d = nc.dram_tensor("beta", [P, MB], F32, kind="ExternalInput")
    out_d = nc.dram_tensor("out", [NL, H1], F32, kind="ExternalOutput")

    with tile.TileContext(nc) as tc:
        with (
            tc.tile_pool(name="big", bufs=1) as big,
            tc.tile_pool(name="slab", bufs=2) as slabp,
            tc.tile_pool(name="io", bufs=4) as iop,
            tc.tile_pool(name="bstream", bufs=3) as bsp,
            tc.tile_pool(name="persist", bufs=4) as pp,
            tc.tile_pool(name="bn", bufs=4) as bnp,
            tc.tile_pool(name="ps", bufs=4, space="PSUM") as psp,
            tc.tile_pool(name="dram", bufs=1, space="DRAM") as dramp,
        ):
            # ---------------- Phase A: h_c = x_rows @ Wall ----------------
            xt_sb = big.tile([P, KB_A, NL], F32R, tag="xt")
            nc.sync.dma_start(
                out=xt_sb,
                in_=xt_d[:, :].rearrange("(kb p) m -> p kb m", p=P),
            )
            wall_t = wall_d[:, :].rearrange("(kb p) j -> p kb j", p=P)

            h_c = dramp.tile([NL, R * H0], F32R, tag="h_c")
            h_all = dramp.tile([N, R * H0], F32R, tag="h_all",
                               addr_space="Shared")

            rootf = []
            for n in range(NBLK):
                ps_n = [psp.tile([P, H0], F32, tag="psA", name=f"psA_{n}_{m}") for m in range(MB)]
                for kh in range(2):
                    slab = slabp.tile([P, KB_A // 2, H0], F32R, tag="slab")
                    nc.sync.dma_start(
                        out=slab,
                        in_=wall_t[:, kh * 16:(kh + 1) * 16,
                                   n * H0:(n + 1) * H0],
                    )
                    for k in range(KB_A // 2):
                        kb = kh * 16 + k
                        for m in range(MB):
                            nc.tensor.matmul(
                                ps_n[m],
                                xt_sb[:, kb, m * P:(m + 1) * P],
                                slab[:, k, :],
                                start=(kb == 0),
                                stop=(kb == KB_A - 1),
                            )
                for m in range(MB):
                    if n == NBLK - 1:
                        # root block: keep local in fp32 (these rows are
                        # exactly this core's dst rows)
                        rf = pp.tile([P, H0], F32, tag="rootf")
                        nc.vector.tensor_copy(out=rf, in_=ps_n[m])
                        rootf.append(rf)
                    else:
                        hsb = iop.tile([P, H0], F32R, tag="hout")
                        nc.vector.tensor_copy(out=hsb, in_=ps_n[m])
                        nc.sync.dma_start(
                            out=h_c[m * P:(m + 1) * P, n * H0:(n + 1) * H0],
                            in_=hsb,
                        )

            nc.gpsimd.collective_compute(
                "AllGather",
                mybir.AluOpType.bypass,
                replica_groups=[list(range(NCORES))],
                ins=[h_c[:, :]],
                outs=[h_all[:, :]],
            )

            # ---------------- Phase B: agg = AT_c.T-contract @ h ----------
            psB = [psp.tile([P, H0], F32, tag="psB", name=f"psB_{m}") for m in range(MB)]
            for kb in range(KB_B):
                r, sb_i = divmod(kb, KB_A)
                ht = bsp.tile([P, H0], F32R, tag="ht")
                nc.sync.dma_start(
                    out=ht,
                    in_=h_all[sb_i * P:(sb_i + 1) * P, r * H0:(r + 1) * H0],
                )
                att = bsp.tile([P, NL], F32R, tag="att")
                nc.sync.dma_start(out=att, in_=at_d[kb * P:(kb + 1) * P, :])
                for m in range(MB):
                    nc.tensor.matmul(
                        psB[m],
                        att[:, m * P:(m + 1) * P],
                        ht,
                        start=(kb == 0),
                        stop=(kb == KB_B - 1),
                    )

            # ---------------- Phase C: feats -> fc -> BN -> ReLU ----------
            fcw_sb = big.tile([QS, QB, H1], F32, tag="fcw")
            nc.sync.dma_start(
                out=fcw_sb,
                in_=fcwt_d[:, :].rearrange("(q p) j -> p q j", p=QS),
            )
            ident = big.tile([P, P], F32, tag="ident")
            make_identity(nc, ident)
            biasb = big.tile([P, H0], F32, tag="bias")
            nc.sync.dma_start(out=biasb, in_=biasb_d[:, :])
            gam = big.tile([P, MB], F32, tag="gam")
            nc.sync.dma_start(out=gam, in_=gamma_d[:, :])
            bet = big.tile([P, MB], F32, tag="bet")
            nc.sync.dma_start(out=bet, in_=beta_d[:, :])
            eps_t = big.tile([P, 1], F32, tag="eps")
            nc.vector.memset(eps_t, EPS)

            feats = []
            for m in range(MB):
                f = pp.tile([P, H0], F32, tag="feats")
                nc.vector.tensor_add(out=f, in0=psB[m], in1=rootf[m])
                nc.vector.tensor_add(out=f, in0=f, in1=biasb)
                feats.append(f)

            fT = [pp.tile([P, NL], F32, tag="fT", name=f"fT_{q}") for q in range(QB)]
            for m in range(MB):
                for q in range(QB):
                    pt = psp.tile([P, P], F32, tag="psA")
                    nc.tensor.transpose(
                        pt[:QS, :], feats[m][:, q * QS:(q + 1) * QS], ident
                    )
                    nc.vector.tensor_copy(
                        out=fT[q][:QS, m * P:(m + 1) * P], in_=pt[:QS, :]
                    )

            for m in range(MB):
                pz = psp.tile([P, H1], F32, tag="psA")
                for q in range(QB):
                    nc.tensor.matmul(
                        pz,
                        fT[q][:QS, m * P:(m + 1) * P],
                        fcw_sb[:, q, :],
                        start=(q == 0),
                        stop=(q == QB - 1),
                    )
                stats = bnp.tile([P, 6], F32, tag="stats")
                nc.vector.bn_stats(out=stats, in_=pz)
                mv = bnp.tile([P, 2], F32, tag="mv")
                nc.vector.bn_aggr(out=mv, in_=stats)
                rstd = bnp.tile([P, 1], F32, tag="rstd")
                nc.scalar.activation(
                    out=rstd, in_=mv[:, 1:2],
                    func=mybir.ActivationFunctionType.Sqrt,
                    bias=eps_t, scale=1.0,
                )
                nc.vector.reciprocal(out=rstd, in_=rstd)
                g2 = bnp.tile([P, 1], F32, tag="g2")
                nc.vector.tensor_mul(out=g2, in0=rstd, in1=gam[:, m:m + 1])
                zt = bnp.tile([P, H1], F32, tag="zt")
                nc.vector.tensor_scalar(
                    out=zt, in0=pz,
                    scalar1=mv[:, 0:1], scalar2=g2,
                    op0=mybir.AluOpType.subtract, op1=mybir.AluOpType.mult,
                )
                nc.scalar.activation(
                    out=zt, in_=zt,
                    func=mybir.ActivationFunctionType.Relu,
                    bias=bet[:, m:m + 1], scale=1.0,
                )
                nc.sync.dma_start(out=out_d[m * P:(m + 1) * P, :], in_=zt)

    nc.finalize()
    return nc


def _get_nc():
    global _NC_CACHE
    if _NC_CACHE is None:
        _NC_CACHE = _build()
    return _NC_CACHE


def kernel(**inputs) -> np.ndarray:
    global LAST_RESULTS
    x = np.asarray(inputs["x"], dtype=np.float32)
    basis = np.asarray(inputs["basis"], dtype=np.float32)
    comp = np.asarray(inputs["comp"], dtype=np.float32)
    root = np.asarray(inputs["root"], dtype=np.float32)
    bias_rgcn = np.asarray(inputs["bias_rgcn"], dtype=np.float32)
    fc_w = np.asarray(inputs["fc_w"], dtype=np.float32)
    bn_gamma_u = np.asarray(inputs["bn_gamma_u"], dtype=np.float32)
    bn_beta_u = np.asarray(inputs["bn_beta_u"], dtype=np.float32)
    bn_gamma_i = np.asarray(inputs["bn_gamma_i"], dtype=np.float32)
    bn_beta_i = np.asarray(inputs["bn_beta_i"], dtype=np.float32)
    edge_index = np.asarray(inputs["edge_index"]).astype(np.int64)
    edge_type = np.asarray(inputs["edge_type"]).astype(np.int64)

    # ---- host prep (index/layout work + weight folding) ----
    src, dst = edge_index[0], edge_index[1]
    et = edge_type

    # W[r] = sum_b comp[r,b] basis[b]; Wall = [W | root]
    W = np.tensordot(comp, basis, axes=([1], [0]))          # [R, N, H0]
    wall = np.empty((N, WCOL), dtype=np.float32)
    wall[:, :R * H0] = W.transpose(1, 0, 2).reshape(N, R * H0)
    wall[:, R * H0:] = root
    wall = round_fp32r(wall)

    xT = round_fp32r(x.T)                                   # [N(i), N(s)]

    # normalized adjacency, transposed: AT[(r*N+src), dst] = count/cnt[dst,r]
    cnt = np.bincount(dst * R + et, minlength=N * R).astype(np.float64)
    w_e = 1.0 / np.maximum(cnt[dst * R + et], 1.0)
    lin = (et * N + src) * np.int64(N) + dst
    at_full = np.bincount(lin, weights=w_e, minlength=R * N * N)
    at_full = at_full.astype(np.float32).reshape(R * N, N)

    fcwt = np.ascontiguousarray(fc_w.T)                     # [H0, H1]
    biasb = np.ascontiguousarray(
        np.broadcast_to(bias_rgcn, (P, H0)), dtype=np.float32)
    gamma_all = np.concatenate([bn_gamma_u, bn_gamma_i])
    beta_all = np.concatenate([bn_beta_u, bn_beta_i])

    in_maps = []
    for c in range(NCORES):
        sl = slice(c * NL, (c + 1) * NL)
        in_maps.append({
            "xt": np.ascontiguousarray(xT[:, sl]),
            "wall": wall,
            "at": round_fp32r(at_full[:, sl]),
            "fcwt": fcwt,
            "biasb": biasb,
            "gamma": np.ascontiguousarray(
                gamma_all[sl].reshape(MB, P).T),
            "beta": np.ascontiguousarray(
                beta_all[sl].reshape(MB, P).T),
        })

    nc = _get_nc()
    res = run_bass_kernel_spmd(
        nc, in_maps, core_ids=list(range(NCORES)), trace=TRACE,
    )
    LAST_RESULTS = res

    z = np.concatenate([res.results[c]["out"] for c in range(NCORES)], axis=0)
    return np.stack([z[:U], z[U:]], axis=0)


# revision 3
# speedup vs baseline: 1.8046x; 1.8046x over previous
"""GCEncoder (RGCN basis-decomposition conv + mean aggregation + Dense/BN/ReLU)
as a Bass/Tile kernel on 8 Trainium2 NeuronCores.

Math (reference):
  W[r]  = sum_b comp[r,b] * basis[b]                    [R, N, H0]
  h[r]  = x @ W[r]                                      [R, N, H0]
  agg[d] = sum_r (1/cnt[d,r]) * sum_{e: dst=d, type=r} h[r, src_e]
  feats = agg + x @ root + bias
  z     = feats @ fc_w.T ; per-row batchnorm over H1 + gamma/beta + relu
  out   = (z[:U], z[U:]) stacked -> [2, U, H1]

Device strategy (per core c of 8, 512 node-rows each):
  Phase A: h rows for this core's 512 src rows: h_c = x[rows] @ Wall where
           Wall = [W[0] | ... | W[4] | root]  (4096 x 3000).  The root block
           result stays local in fp32 (these rows are exactly this core's dst
           rows); each relation block r is AllGathered as soon as it is done
           (5 chunked collectives overlap with the remaining compute).
  Phase B: agg rows via dense normalized-adjacency matmul: contraction over
           the 20480 (r,src) axis with host-built AT[(r,src), dst_local],
           PSUM-accumulated across 160 k-tiles into 4 persistent banks.
  Phase C: feats = agg + root_part + bias; PE-transpose; z = feats @ fc_w.T;
           per-row BN (bn_stats/bn_aggr) + gamma/beta + ReLU.

Matmul operands are bf16 (fp32 PSUM accumulation); set USE_FP32R=True for
E8M11 fp32r operands instead (2x slower matmul stream + 2x DMA, ~15x lower
error).  All heavy inputs are host-pre-swizzled so each DMA lands >=4KB
contiguous per SBUF partition.
"""
import numpy as np
import ml_dtypes

import concourse.bacc as bacc
import concourse.mybir as mybir
import concourse.tile as tile
from concourse.bass_utils import run_bass_kernel_spmd
from concourse.masks import make_identity

P = 128
NCORES = 8
N = 4096          # nodes
U = 2048          # users
R = 5             # relations
H0 = 500
H1 = 75
EPS = 1e-5

NL = N // NCORES              # 512 node rows per core
KB_A = N // P                 # 32 contraction tiles, phase A
WCOL = R * H0 + H0            # 3000 Wall columns
NBLK = WCOL // H0             # 6 column blocks of 500
MB = NL // P                  # 4 M-tiles per core
QB = 4                        # H0 chunks for transpose/fc
QS = H0 // QB                 # 125

F32 = mybir.dt.float32

USE_FP32R = False
if USE_FP32R:
    DT_MM = mybir.dt.float32r
else:
    DT_MM = mybir.dt.bfloat16

# test hooks
TRACE = False
LAST_RESULTS = None
_NC_CACHE = None


def round_fp32r(a: np.ndarray) -> np.ndarray:
    """Round fp32 to fp32r (E8M11): RNE at mantissa bit 12, low 12 bits zero."""
    b = np.ascontiguousarray(a, dtype=np.float32).view(np.uint32).astype(np.uint64)
    b = b + 0x7FF + ((b >> 12) & 1)
    return (b & 0xFFFFF000).astype(np.uint32).view(np.float32)


def _prep_mm(a: np.ndarray) -> np.ndarray:
    """Convert host fp32 data to the matmul operand dtype."""
    if USE_FP32R:
        return round_fp32r(a)
    return np.ascontiguousarray(a).astype(ml_dtypes.bfloat16)


def _build():
    nc = bacc.Bacc("TRN2", target_bir_lowering=False, debug=False,
                   num_devices=NCORES)

    # host-swizzled inputs; layouts noted as [partition, free...]
    # x4[p, kb*NL + m] = x[coreRows m][i = kb*128+p]
    x4_d = nc.dram_tensor("x4", [P, KB_A * NL], DT_MM, kind="ExternalInput")
    # w4[p, ((n*32+kb) * H0) + j] = Wall[kb*128+p, n*500+j]
    w4_d = nc.dram_tensor("w4", [P, NBLK * KB_A * H0], DT_MM,
                          kind="ExternalInput")
    # a4[p, kb*NL + d] = AT[kb*128+p, d]   (kb = r*32 + cb*4 + mk)
    a4_d = nc.dram_tensor("a4", [P, R * KB_A * NL], DT_MM,
                          kind="ExternalInput")
    fcwt_d = nc.dram_tensor("fcwt", [H0, H1], F32, kind="ExternalInput")
    biasb_d = nc.dram_tensor("biasb", [P, H0], F32, kind="ExternalInput")
    gamma_d = nc.dram_tensor("gamma", [P, MB], F32, kind="ExternalInput")
    beta_d = nc.dram_tensor("beta", [P, MB], F32, kind="ExternalInput")
    out_d = nc.dram_tensor("out", [NL, H1], F32, kind="ExternalOutput")

    with tile.TileContext(nc) as tc:
        with (
            tc.tile_pool(name="big", bufs=1) as big,
            tc.tile_pool(name="slab", bufs=2) as slabp,
            tc.tile_pool(name="io", bufs=4) as iop,
            tc.tile_pool(name="bstream", bufs=3) as bsp,
            tc.tile_pool(name="persist", bufs=4) as pp,
            tc.tile_pool(name="bn", bufs=4) as bnp,
            tc.tile_pool(name="ps", bufs=4, space="PSUM") as psp,
            tc.tile_pool(name="dram", bufs=1, space="DRAM") as dramp,
        ):
            # ---------------- Phase A: h_c = x_rows @ Wall ----------------
            xt_sb = big.tile([P, KB_A, NL], DT_MM, tag="xt")
            for ch in range(4):
                nc.sync.dma_start(
                    out=xt_sb[:, ch * 8:(ch + 1) * 8, :],
                    in_=x4_d[:, ch * 8 * NL:(ch + 1) * 8 * NL],
                )

            # per-relation h buffers: h_cr[p, m*500+j]; gathered to
            # h_ar[128*rank + p, m*500+j]
            h_cr = [dramp.tile([P, MB * H0], DT_MM, tag="h_c", name=f"h_c{r}")
                    for r in range(R)]
            h_ar = [dramp.tile([NCORES * P, MB * H0], DT_MM, tag="h_a",
                               addr_space="Shared", name=f"h_a{r}")
                    for r in range(R)]

            rootf = []
            for n in range(NBLK):
                ps_n = [psp.tile([P, H0], F32, tag="psA",
                                 name=f"psA_{n}_{m}") for m in range(MB)]
                for kh in range(2):
                    slab = slabp.tile([P, KB_A // 2, H0], DT_MM, tag="slab")
                    base = (n * KB_A + kh * 16) * H0
                    nc.sync.dma_start(
                        out=slab,
                        in_=w4_d[:, base:base + 16 * H0],
                    )
                    for k in range(KB_A // 2):
                        kb = kh * 16 + k
                        for m in range(MB):
                            nc.tensor.matmul(
                                ps_n[m],
                                xt_sb[:, kb, m * P:(m + 1) * P],
                                slab[:, k, :],
                                start=(kb == 0),
                                stop=(kb == KB_A - 1),
                            )
                for m in range(MB):
                    if n == NBLK - 1:
                        rf = pp.tile([P, H0], F32, tag="rootf",
                                     name=f"rootf_{m}")
                        nc.vector.tensor_copy(out=rf, in_=ps_n[m])
                        rootf.append(rf)
                    else:
                        hsb = iop.tile([P, H0], DT_MM, tag="hout")
                        nc.vector.tensor_copy(out=hsb, in_=ps_n[m])
                        nc.sync.dma_start(
                            out=h_cr[n][:, m * H0:(m + 1) * H0],
                            in_=hsb,
                        )
                if n < R:
                    nc.gpsimd.collective_compute(
                        "AllGather",
                        mybir.AluOpType.bypass,
                        replica_groups=[list(range(NCORES))],
                        ins=[h_cr[n][:, :]],
                        outs=[h_ar[n][:, :]],
                    )

            # ---------------- Phase B: agg = AT.T-contract @ h ------------
            psB = [psp.tile([P, H0], F32, tag="psB", name=f"psB_{m}")
                   for m in range(MB)]
            for r in range(R):
                for cb in range(NCORES):
                    hh = bsp.tile([P, MB * H0], DT_MM, tag="hh")
                    nc.sync.dma_start(
                        out=hh, in_=h_ar[r][cb * P:(cb + 1) * P, :]
                    )
                    aa = bsp.tile([P, MB, NL], DT_MM, tag="aa")
                    base = (r * KB_A + cb * MB) * NL
                    nc.sync.dma_start(
                        out=aa, in_=a4_d[:, base:base + MB * NL]
                    )
                    first = (r == 0 and cb == 0)
                    last = (r == R - 1 and cb == NCORES - 1)
                    for mk in range(MB):
                        for m in range(MB):
                            nc.tensor.matmul(
                                psB[m],
                                aa[:, mk, m * P:(m + 1) * P],
                                hh[:, mk * H0:(mk + 1) * H0],
                                start=(first and mk == 0),
                                stop=(last and mk == MB - 1),
                            )

            # ---------------- Phase C: feats -> fc -> BN -> ReLU ----------
            fcw_sb = big.tile([QS, QB, H1], F32, tag="fcw")
            nc.sync.dma_start(
                out=fcw_sb,
                in_=fcwt_d[:, :].rearrange("(q p) j -> p q j", p=QS),
            )
            ident = big.tile([P, P], F32, tag="ident")
            make_identity(nc, ident)
            biasb = big.tile([P, H0], F32, tag="bias")
            nc.sync.dma_start(out=biasb, in_=biasb_d[:, :])
            gam = big.tile([P, MB], F32, tag="gam")
            nc.sync.dma_start(out=gam, in_=gamma_d[:, :])
            bet = big.tile([P, MB], F32, tag="bet")
            nc.sync.dma_start(out=bet, in_=beta_d[:, :])
            eps_t = big.tile([P, 1], F32, tag="eps")
            nc.vector.memset(eps_t, EPS)

            feats = []
            for m in range(MB):
                f = pp.tile([P, H0], F32, tag="feats", name=f"feats_{m}")
                nc.vector.tensor_add(out=f, in0=psB[m], in1=rootf[m])
                nc.vector.tensor_add(out=f, in0=f, in1=biasb)
                feats.append(f)

            fT = [pp.tile([P, NL], F32, tag="fT", name=f"fT_{q}")
                  for q in range(QB)]
            for m in range(MB):
                for q in range(QB):
                    pt = psp.tile([P, P], F32, tag="psA", name=f"pt_{m}_{q}")
                    nc.tensor.transpose(
                        pt[:QS, :], feats[m][:, q * QS:(q + 1) * QS], ident
                    )
                    nc.vector.tensor_copy(
                        out=fT[q][:QS, m * P:(m + 1) * P], in_=pt[:QS, :]
                    )

            for m in range(MB):
                pz = psp.tile([P, H1], F32, tag="psA", name=f"pz_{m}")
                for q in range(QB):
                    nc.tensor.matmul(
                        pz,
                        fT[q][:QS, m * P:(m + 1) * P],
                        fcw_sb[:, q, :],
                        start=(q == 0),
                        stop=(q == QB - 1),
                    )
                stats = bnp.tile([P, 6], F32, tag="stats")
                nc.vector.bn_stats(out=stats, in_=pz)
                mv = bnp.tile([P, 2], F32, tag="mv")
                nc.vector.bn_aggr(out=mv, in_=stats)
                rstd = bnp.tile([P, 1], F32, tag="rstd")
                nc.scalar.activation(
                    out=rstd, in_=mv[:, 1:2],
                    func=mybir.ActivationFunctionType.Sqrt,
                    bias=eps_t, scale=1.0,
                )
                nc.vector.reciprocal(out=rstd, in_=rstd)
                g2 = bnp.tile([P, 1], F32, tag="g2")
                nc.vector.tensor_mul(out=g2, in0=rstd, in1=gam[:, m:m + 1])
                zt = bnp.tile([P, H1], F32, tag="zt")
                nc.vector.tensor_scalar(
                    out=zt, in0=pz,
                    scalar1=mv[:, 0:1], scalar2=g2,
                    op0=mybir.AluOpType.subtract, op1=mybir.AluOpType.mult,
                )
                nc.scalar.activation(
                    out=zt, in_=zt,
                    func=mybir.ActivationFunctionType.Relu,
                    bias=bet[:, m:m + 1], scale=1.0,
                )
                nc.sync.dma_start(out=out_d[m * P:(m + 1) * P, :], in_=zt)

    nc.finalize()
    return nc


def _get_nc():
    global _NC_CACHE
    if _NC_CACHE is None:
        _NC_CACHE = _build()
    return _NC_CACHE


def kernel(**inputs) -> np.ndarray:
    global LAST_RESULTS
    x = np.asarray(inputs["x"], dtype=np.float32)
    basis = np.asarray(inputs["basis"], dtype=np.float32)
    comp = np.asarray(inputs["comp"], dtype=np.float32)
    root = np.asarray(inputs["root"], dtype=np.float32)
    bias_rgcn = np.asarray(inputs["bias_rgcn"], dtype=np.float32)
    fc_w = np.asarray(inputs["fc_w"], dtype=np.float32)
    bn_gamma_u = np.asarray(inputs["bn_gamma_u"], dtype=np.float32)
    bn_beta_u = np.asarray(inputs["bn_beta_u"], dtype=np.float32)
    bn_gamma_i = np.asarray(inputs["bn_gamma_i"], dtype=np.float32)
    bn_beta_i = np.asarray(inputs["bn_beta_i"], dtype=np.float32)
    edge_index = np.asarray(inputs["edge_index"]).astype(np.int64)
    edge_type = np.asarray(inputs["edge_type"]).astype(np.int64)

    src, dst = edge_index[0], edge_index[1]
    et = edge_type

    # W[r] = sum_b comp[r,b] basis[b]; Wall = [W | root]
    W = np.tensordot(comp, basis, axes=([1], [0]))          # [R, N, H0]
    wall = np.empty((N, WCOL), dtype=np.float32)
    wall[:, :R * H0] = W.transpose(1, 0, 2).reshape(N, R * H0)
    wall[:, R * H0:] = root
    wall16 = _prep_mm(wall)
    # w4[p, (n*32+kb)*H0 + j] = wall[kb*128+p, n*500+j]
    w4 = np.ascontiguousarray(
        wall16.reshape(KB_A, P, NBLK, H0)       # [kb, p, n, j]
        .transpose(1, 2, 0, 3)                  # [p, n, kb, j]
        .reshape(P, NBLK * KB_A * H0))

    xT16 = _prep_mm(x.T)                                    # [i, s]
    # x4[p, kb*NL + m] = x.T[kb*128+p, m@core]  (per-core slice below)
    x4_full = (xT16.reshape(KB_A, P, N)         # [kb, p, s]
               .transpose(1, 0, 2))             # [p, kb, s]

    # normalized adjacency transposed: AT[(r*N+src), dst] = count/cnt[dst,r]
    cnt = np.bincount(dst * R + et, minlength=N * R).astype(np.float64)
    w_e = 1.0 / np.maximum(cnt[dst * R + et], 1.0)
    lin = (et * N + src) * np.int64(N) + dst
    at_full = np.bincount(lin, weights=w_e, minlength=R * N * N)
    at_full = _prep_mm(at_full.astype(np.float32).reshape(R * N, N))
    # a4[p, kb*NL + d] = AT[kb*128+p, d]
    a4_full = (at_full.reshape(R * KB_A, P, N)  # [kb, p, d]
               .transpose(1, 0, 2))             # [p, kb, d]

    fcwt = np.ascontiguousarray(fc_w.T)
    biasb = np.ascontiguousarray(
        np.broadcast_to(bias_rgcn, (P, H0)), dtype=np.float32)
    gamma_all = np.concatenate([bn_gamma_u, bn_gamma_i])
    beta_all = np.concatenate([bn_beta_u, bn_beta_i])

    in_maps = []
    for c in range(NCORES):
        sl = slice(c * NL, (c + 1) * NL)
        in_maps.append({
            "x4": np.ascontiguousarray(
                x4_full[:, :, sl]).reshape(P, KB_A * NL),
            "w4": w4,
            "a4": np.ascontiguousarray(
                a4_full[:, :, sl]).reshape(P, R * KB_A * NL),
            "fcwt": fcwt,
            "biasb": biasb,
            "gamma": np.ascontiguousarray(gamma_all[sl].reshape(MB, P).T),
            "beta": np.ascontiguousarray(beta_all[sl].reshape(MB, P).T),
        })

    nc = _get_nc()
    res = run_bass_kernel_spmd(
        nc, in_maps, core_ids=list(range(NCORES)), trace=TRACE,
    )
    LAST_RESULTS = res

    z = np.concatenate([res.results[c]["out"] for c in range(NCORES)], axis=0)
    return np.stack([z[:U], z[U:]], axis=0)
